# revision 1
# baseline (speedup 1.0000x reference)
"""Trainium2 Bass kernel for the L1Writer scatter-memory problem.

Computes   out = 0.95 * memory + einsum('bs,bshk,bshv->hkv', rho, keys, values)

Strategy: data-parallel over the flattened (B*S)=16384 token axis, 2048 rows
per core.  Each core computes its partial delta
    delta_h = K_h^T diag(rho) V_h        (per head h, shapes (2048,64))
as a chain of 128-row PE matmuls accumulating in PSUM.  The 8 partial
(H,Dk,Dv) deltas are summed on the host (tiny: 256 KB each) and added to
decay*memory there.

Per-core kernel layout:
  - keys/values arrive as (2048, 1024) row-major shards; loaded in 4 mega
    tiles of [128 partitions x 4096 fp32] (2 MB DMAs, 4 KB contiguous runs).
  - rho arrives pre-transposed as (128, 16): partition p, chunk c -> rho of
    token c*128+p.  Keys are scaled by rho on the vector engine
    (per-partition tensor_scalar broadcast).
  - 16 heads accumulate into 2 PSUM banks ([64, 512] each, 8 heads per
    bank).  Banks are zeroed with a DVE memset and every matmul uses
    start=False, so each element's first matmul overwrites (has_written
    unset) or accumulates onto the memset zero (has_written stale-set);
    both give the correct sum without any whole-bank-clear hazards.
  - PSUM -> SBUF copy -> one contiguous 256 KB DMA out in [k, h*64+v]
    layout; the host transposes to (h, k, v).
"""

import numpy as np

DECAY = 0.95
B, S, H, Dk, Dv = 4, 4096, 16, 64, 64
N_CORES = 8
NS = (B * S) // N_CORES          # 2048 rows per core
P = 128                          # partitions
CHUNKS = NS // P                 # 16 contraction chunks of 128 rows
MEGA = 4                         # chunks per DMA mega-tile
N_MEGA = CHUNKS // MEGA          # 4 mega tiles
FD = H * Dk                      # 1024 features per row

_nc_cache = None


def _build_nc():
    from contextlib import ExitStack

    import concourse.bass as bass
    import concourse.mybir as mybir

    f32 = mybir.dt.float32
    nc = bass.Bass()

    keys_d = nc.dram_tensor("keys", (NS, FD), f32, kind="ExternalInput")
    vals_d = nc.dram_tensor("values", (NS, FD), f32, kind="ExternalInput")
    rho_d = nc.dram_tensor("rho", (P, CHUNKS), f32, kind="ExternalInput")
    out_d = nc.dram_tensor("delta", (Dk, H * Dv), f32, kind="ExternalOutput")

    # mega tile m, partition p, free (j, f): row (m*MEGA + j)*128 + p
    keys_r = keys_d.rearrange("(m j p) f -> m p j f", j=MEGA, p=P)
    vals_r = vals_d.rearrange("(m j p) f -> m p j f", j=MEGA, p=P)

    # Raw bass (no Tile): this container's walrus rejects engine
    # instructions carrying >1 attached semaphore wait, so all waits are
    # standalone sequencer wait_ge ops and every hazard is hand-managed.
    #
    # Engine programs:
    #  SP (sync):  rho DMA, then kt[m]/vt[m] mega DMAs (2 MB each,
    #              double-buffered; WAR waits vs DVE/PE for slot reuse),
    #              final out DMA.
    #  DVE:        memset both PSUM accumulators, then per (m,j) scale keys
    #              by rho (per-partition tensor_scalar), finally evacuate
    #              PSUM -> SBUF.
    #  PE:         per (m,j): 16 head matmuls accumulating into 2 PSUM
    #              banks (8 heads x 64 cols each); all start=False onto
    #              memset zeros (first write per element overwrites or
    #              adds to zero -- correct for any stale has_written bits).
    #
    # dve_sem increments: 1 (memsets) + 16 (scales) + 2 (evac) = 19
    # pe_sem increments: 1 per (m,j) group = 16
    with ExitStack() as ctx:
        kt = [
            ctx.enter_context(nc.sbuf_tensor(f"kt{i}", [P, MEGA, FD], f32))
            for i in range(2)
        ]
        vt = [
            ctx.enter_context(nc.sbuf_tensor(f"vt{i}", [P, MEGA, FD], f32))
            for i in range(2)
        ]
        kts = [
            ctx.enter_context(nc.sbuf_tensor(f"kts{i}", [P, MEGA, FD], f32))
            for i in range(2)
        ]
        rho_t = ctx.enter_context(nc.sbuf_tensor("rho_t", [P, CHUNKS], f32))
        out_t = ctx.enter_context(nc.sbuf_tensor("out_t", [Dk, H * Dv], f32))
        acc = [
            ctx.enter_context(nc.psum_tensor(f"acc{i}", [Dk, 8 * Dv], f32))
            for i in range(2)
        ]
        rs = ctx.enter_context(nc.semaphore(name="rs"))
        ks = [ctx.enter_context(nc.semaphore(name=f"ks{i}")) for i in range(N_MEGA)]
        vs = [ctx.enter_context(nc.semaphore(name=f"vs{i}")) for i in range(N_MEGA)]
        dve_sem = ctx.enter_context(nc.semaphore(name="dve_sem"))
        out_sem = ctx.enter_context(nc.semaphore(name="out_sem"))
        done_sem = ctx.enter_context(nc.semaphore(name="done_sem"))
        pe_sem = ctx.enter_context(nc.semaphore(name="pe_sem"))
        block = ctx.enter_context(nc.Block())

        @block.sync
        def _(sync):
            sync.dma_start(rho_t[:], rho_d[:]).then_inc(rs, 16)
            for m in range(N_MEGA):
                if m >= 2:
                    # kt slot WAR: scales of m-2 done (1 + (m-2)*4 + 4)
                    sync.wait_ge(dve_sem, (m - 2) * 4 + 5)
                sync.dma_start(kt[m % 2][:], keys_r[m]).then_inc(ks[m], 16)
                if m >= 2:
                    # vt slot WAR: matmul groups of m-2 done
                    sync.wait_ge(pe_sem, (m - 2) * 4 + 4)
                sync.dma_start(vt[m % 2][:], vals_r[m]).then_inc(vs[m], 16)
            sync.wait_ge(dve_sem, 19)
            sync.dma_start(out_d[:], out_t[:]).then_inc(out_sem, 16)
            sync.wait_ge(out_sem, 16)
            sync.nop().then_inc(done_sem, 1)

        @block.gpsimd
        def _(gpsimd):
            # Semaphores persist across NEFF executions; clear them all at
            # the end (after every engine is provably done) so the kernel
            # is safe to run repeatedly.
            gpsimd.wait_ge(done_sem, 1)
            for s in [rs, *ks, *vs, dve_sem, pe_sem, out_sem, done_sem]:
                gpsimd.sem_clear(s)

        @block.vector
        def _(vector):
            vector.memset(acc[0][:], 0.0)
            vector.memset(acc[1][:], 0.0).then_inc(dve_sem, 1)
            vector.wait_ge(rs, 16)
            for m in range(N_MEGA):
                vector.wait_ge(ks[m], 16)
                if m >= 2:
                    # kts slot WAR: matmul groups of m-2 done
                    vector.wait_ge(pe_sem, (m - 2) * 4 + 4)
                for j in range(MEGA):
                    c = m * MEGA + j
                    vector.tensor_scalar_mul(
                        kts[m % 2][:, j, :],
                        kt[m % 2][:, j, :],
                        rho_t[:, c : c + 1],
                    ).then_inc(dve_sem, 1)
            vector.wait_ge(pe_sem, 16)
            for g in range(2):
                vector.tensor_copy(
                    out_t[:, g * 512 : (g + 1) * 512], acc[g][:]
                ).then_inc(dve_sem, 1)

        @block.tensor
        def _(tensor):
            for m in range(N_MEGA):
                tensor.wait_ge(vs[m], 16)
                for j in range(MEGA):
                    # memsets + scales up to (m,j) done
                    tensor.wait_ge(dve_sem, m * 4 + j + 2)
                    for h in range(H):
                        g, hh = divmod(h, 8)
                        mm = tensor.matmul(
                            acc[g][:, hh * Dv : (hh + 1) * Dv],
                            kts[m % 2][:, j, h * Dk : (h + 1) * Dk],
                            vt[m % 2][:, j, h * Dv : (h + 1) * Dv],
                            start=False,
                            stop=(m == N_MEGA - 1 and j == MEGA - 1),
                            skip_group_check=True,
                        )
                        if h == H - 1:
                            mm.then_inc(pe_sem, 1)

    return nc


def _get_nc():
    global _nc_cache
    if _nc_cache is None:
        _nc_cache = _build_nc()
    return _nc_cache


def _make_in_maps(keys, values, write_strengths):
    kf = np.ascontiguousarray(keys.reshape(B * S, FD))
    vf = np.ascontiguousarray(values.reshape(B * S, FD))
    wf = np.asarray(write_strengths).reshape(B * S)
    in_maps = []
    for c in range(N_CORES):
        sl = slice(c * NS, (c + 1) * NS)
        in_maps.append(
            {
                "keys": np.ascontiguousarray(kf[sl]),
                "values": np.ascontiguousarray(vf[sl]),
                "rho": np.ascontiguousarray(wf[sl].reshape(CHUNKS, P).T),
            }
        )
    return in_maps


def _run(in_maps, **kwargs):
    from concourse.bass_utils import run_bass_kernel_spmd

    nc = _get_nc()
    return run_bass_kernel_spmd(nc, in_maps, core_ids=list(range(N_CORES)), **kwargs)


def _assemble(memory, results):
    parts = np.stack([r["delta"] for r in results], axis=0)  # (8, 64, 1024)
    delta = parts.sum(axis=0, dtype=np.float64)  # (64, 1024) in [k, h*64+v]
    delta_hkv = delta.reshape(Dk, H, Dv).transpose(1, 0, 2)  # (H, Dk, Dv)
    out = DECAY * np.asarray(memory, dtype=np.float64) + delta_hkv
    return out.astype(np.float32)


def kernel(memory, keys, values, write_strengths):
    memory = np.asarray(memory, dtype=np.float32)
    keys = np.asarray(keys, dtype=np.float32)
    values = np.asarray(values, dtype=np.float32)
    write_strengths = np.asarray(write_strengths, dtype=np.float32)

    in_maps = _make_in_maps(keys, values, write_strengths)
    res = _run(in_maps)
    return _assemble(memory, res.results)


if __name__ == "__main__":
    rng = np.random.default_rng(0)
    mem = rng.standard_normal((H, Dk, Dv), dtype=np.float32)
    k = rng.standard_normal((B, S, H, Dk), dtype=np.float32)
    v = rng.standard_normal((B, S, H, Dv), dtype=np.float32)
    w = rng.random((B, S), dtype=np.float32)
    out = kernel(mem, k, v, w)
    ref = DECAY * mem + np.einsum(
        "bs,bshk,bshv->hkv", w.astype(np.float64), k.astype(np.float64), v.astype(np.float64)
    )
    err = np.abs(out - ref).max() / np.abs(ref).max()
    print("self-check rel err:", err)



# revision 21
# speedup vs baseline: 1.0636x; 1.0636x over previous
"""Trainium2 Bass kernel for the L1Writer scatter-memory problem.

Computes   out = 0.95 * memory + einsum('bs,bshk,bshv->hkv', rho, keys, values)

Strategy: data-parallel over the flattened (B*S)=16384 token axis, 2048 rows
per core.  Each core computes its partial delta
    delta_h = K_h^T diag(rho) V_h        (per head h, contraction over 2048)
as a chain of 128-row PE matmuls accumulating in PSUM.  The 8 partial
(H,Dk,Dv) deltas are summed on the host (tiny: 256 KB each) and added to
decay*memory there.

Per-core pipeline (memory-bound: 16 MB of f32 K/V per core, ~40 us at the
~400+ GB/s per-core streaming rate):
  - K/V stream interleaved per 512 KB chunk (128 partitions x 4 KB
    contiguous descriptors) on the SP (sync) HWDGE queue.  rho goes first
    on the same queue (small scattered descriptors, ~1 us).
  - ACT scales keys by rho (per-partition `scale` AP on an activation
    Copy) casting f32->bf16, and copy-casts values f32->bf16.  bf16
    matmuls run ~3-4x faster on PE than f32, which turns the old 23 us
    PE-bound tail into a few us.
  - 16 heads accumulate into 2 PSUM banks ([64, 512] each, 8 heads per
    bank).  The first matmul touching each bank uses start=True: it
    clears the bank's has_written bits, so every later start=False matmul
    overwrites its region on first touch and accumulates after -- no
    zero-fill needed and no stale state across NEFF reruns.
  - ACT also evacuates PSUM -> SBUF at the end, then one contiguous
    256 KB DMA out in [k, h*64+v] layout; the host transposes to
    (h, k, v).

Hard-won hazard notes (HW-measured, this container):
  - One DMA per semaphore threshold.  N DMAs bumping one semaphore by 16
    each with consumers waiting partial thresholds (16*(j+1)) is UNSOUND:
    the 16 SDMA engines complete their per-DMA slices with skew, so
    engines running ahead on later DMAs push the count past 16*(j+1)
    while a laggard is still landing chunk j -> torn reads, sticky
    per-NEFF-load nondeterministic corruption.
  - A DVE memset does not touch PSUM has_written bits and engine start is
    skewed by iram loads, so memset-then-accumulate races the first
    matmuls.  start=True on each bank's first matmul replaces it safely.
  - PE drains (fusable=False) before the inc releasing the PSUM
    evacuation: the systolic array writes PSUM ~175 ns after the last
    matmul commits.  Same for ACT before releasing out_t to the out DMA.
"""

import numpy as np

DECAY = 0.95
B, S, H, Dk, Dv = 4, 4096, 16, 64, 64
N_CORES = 8
NS = (B * S) // N_CORES          # 2048 rows per core
P = 128                          # partitions
CHUNKS = NS // P                 # 16 contraction chunks of 128 rows
MEGA = 4                         # chunks per buffer slot group
N_MEGA = CHUNKS // MEGA          # 4 slot groups, double-buffered
FD = H * Dk                      # 1024 features per row

_nc_cache = None


def _build_nc():
    from contextlib import ExitStack

    import concourse.bass as bass
    import concourse.mybir as mybir

    f32 = mybir.dt.float32
    bf16 = mybir.dt.bfloat16
    nc = bass.Bass()

    keys_d = nc.dram_tensor("keys", (NS, FD), f32, kind="ExternalInput")
    vals_d = nc.dram_tensor("values", (NS, FD), f32, kind="ExternalInput")
    # rho is zero-padded to 128 cols (512 B per partition) on the host: at
    # 64 B per partition the 128 scattered descriptors took ~6 us and
    # stalled the whole K/V stream behind them on the queue.
    rho_d = nc.dram_tensor("rho", (P, P), f32, kind="ExternalInput")
    out_d = nc.dram_tensor("delta", (Dk, H * Dv), f32, kind="ExternalOutput")

    # Raw bass (no Tile): this container's walrus rejects engine
    # instructions carrying >1 attached semaphore wait, so all waits are
    # standalone sequencer wait_ge ops and every hazard is hand-managed.
    #
    # Chunk c = m*4+j covers token rows [c*128, (c+1)*128).  kt/vt/ktb/vtb
    # are double-buffered over m (slot m%2), 4 chunks per slot.
    #
    # Semaphore roles:
    #   kcs[c]/vcs[c]: one per chunk DMA, +16 at completion.  One DMA per
    #     semaphore threshold is load-bearing: the 16 SDMA engines complete
    #     their per-DMA slices with skew, so a shared semaphore passed
    #     partial thresholds (16*(j+1)) before chunk j fully landed ->
    #     torn reads.  (Cost: ~30 extra sems; 256 exist per core.)
    #   scale_sem:   +1 per ACT key-scale   -> m*4+j+1 after scale (m,j)
    #   act_sem:     +1 per ACT value-cast  -> m*4+j+1 after cast (m,j)
    #   pe_sem:      +1 per matmul group    -> m*4+j+1 after group (m,j)
    #   evac_sem:    2 after ACT evacuates PSUM to out_t
    with ExitStack() as ctx:
        kt = [
            ctx.enter_context(nc.sbuf_tensor(f"kt{i}", [P, MEGA, FD], f32))
            for i in range(2)
        ]
        vt = [
            ctx.enter_context(nc.sbuf_tensor(f"vt{i}", [P, MEGA, FD], f32))
            for i in range(2)
        ]
        ktb = [
            ctx.enter_context(nc.sbuf_tensor(f"ktb{i}", [P, MEGA, FD], bf16))
            for i in range(2)
        ]
        vtb = [
            ctx.enter_context(nc.sbuf_tensor(f"vtb{i}", [P, MEGA, FD], bf16))
            for i in range(2)
        ]
        rho_t = ctx.enter_context(nc.sbuf_tensor("rho_t", [P, P], f32))
        out_t = ctx.enter_context(nc.sbuf_tensor("out_t", [Dk, H * Dv], f32))
        acc = [
            ctx.enter_context(nc.psum_tensor(f"acc{i}", [Dk, 8 * Dv], f32))
            for i in range(2)
        ]
        rs = ctx.enter_context(nc.semaphore(name="rs"))
        kcs = [ctx.enter_context(nc.semaphore(name=f"kc{i}")) for i in range(CHUNKS)]
        vcs = [ctx.enter_context(nc.semaphore(name=f"vc{i}")) for i in range(CHUNKS)]
        scale_sem = ctx.enter_context(nc.semaphore(name="scale_sem"))
        act_sem = ctx.enter_context(nc.semaphore(name="act_sem"))
        evac_sem = ctx.enter_context(nc.semaphore(name="evac_sem"))
        out_sem = ctx.enter_context(nc.semaphore(name="out_sem"))
        done_sem = ctx.enter_context(nc.semaphore(name="done_sem"))
        pe_sem = ctx.enter_context(nc.semaphore(name="pe_sem"))
        block = ctx.enter_context(nc.Block())

        def chunk_rows(m, j):
            c = m * MEGA + j
            return slice(c * P, (c + 1) * P)

        @block.sync
        def _(sync):
            sync.dma_start(rho_t[:], rho_d[:]).then_inc(rs, 16)
            for m in range(N_MEGA):
                for j in range(MEGA):
                    if m >= 2:
                        # slot WAR: ACT consumed chunk (m-2, j) from kt/vt
                        sync.wait_ge(scale_sem, (m - 2) * MEGA + j + 1)
                        sync.wait_ge(act_sem, (m - 2) * MEGA + j + 1)
                    c = m * MEGA + j
                    sync.dma_start(
                        kt[m % 2][:, j, :], keys_d[chunk_rows(m, j), :]
                    ).then_inc(kcs[c], 16)
                    sync.dma_start(
                        vt[m % 2][:, j, :], vals_d[chunk_rows(m, j), :]
                    ).then_inc(vcs[c], 16)
            sync.wait_ge(evac_sem, 2)
            sync.dma_start(out_d[:], out_t[:]).then_inc(out_sem, 16)
            sync.wait_ge(out_sem, 16)
            sync.nop().then_inc(done_sem, 1)

        @block.scalar
        def _(scalar):
            # value casts f32 -> bf16 (key scales run concurrently on DVE)
            for m in range(N_MEGA):
                for j in range(MEGA):
                    c = m * MEGA + j
                    scalar.wait_ge(vcs[c], 16)
                    if m >= 2:
                        # vtb slot WAR: matmul group (m-2, j) done
                        scalar.wait_ge(pe_sem, (m - 2) * MEGA + j + 1)
                    scalar.copy(vtb[m % 2][:, j, :], vt[m % 2][:, j, :]).then_inc(
                        act_sem, 1
                    )
            # PSUM evacuation
            scalar.wait_ge(pe_sem, 16)
            for g in range(2):
                scalar.copy(out_t[:, g * 512 : (g + 1) * 512], acc[g][:])
            scalar.drain(fusable=False).then_inc(evac_sem, 2)

        @block.vector
        def _(vector):
            # key scales by rho, f32 -> bf16
            vector.wait_ge(rs, 16)
            for m in range(N_MEGA):
                for j in range(MEGA):
                    c = m * MEGA + j
                    vector.wait_ge(kcs[c], 16)
                    if m >= 2:
                        # ktb slot WAR: matmul group (m-2, j) done
                        vector.wait_ge(pe_sem, (m - 2) * MEGA + j + 1)
                    vector.tensor_scalar_mul(
                        ktb[m % 2][:, j, :],
                        kt[m % 2][:, j, :],
                        rho_t[:, c : c + 1],
                    ).then_inc(scale_sem, 1)

        @block.gpsimd
        def _(gpsimd):
            # Semaphores persist across NEFF executions; clear them all at
            # the end (after every engine is provably done) so the kernel
            # is safe to run repeatedly.
            gpsimd.wait_ge(done_sem, 1)
            for s in [
                rs, *kcs, *vcs, scale_sem, act_sem, evac_sem, pe_sem, out_sem,
                done_sem,
            ]:
                gpsimd.sem_clear(s)

        @block.tensor
        def _(tensor):
            for m in range(N_MEGA):
                for j in range(MEGA):
                    tensor.wait_ge(scale_sem, m * MEGA + j + 1)
                    tensor.wait_ge(act_sem, m * MEGA + j + 1)
                    first = m == 0 and j == 0
                    last = m == N_MEGA - 1 and j == MEGA - 1
                    for h in range(H):
                        g, hh = divmod(h, 8)
                        mm = tensor.matmul(
                            acc[g][:, hh * Dv : (hh + 1) * Dv],
                            ktb[m % 2][:, j, h * Dk : (h + 1) * Dk],
                            vtb[m % 2][:, j, h * Dv : (h + 1) * Dv],
                            # first touch of each bank clears its
                            # has_written bits; later matmuls overwrite
                            # untouched regions and accumulate touched ones
                            start=(first and hh == 0),
                            stop=last,
                            skip_group_check=True,
                        )
                        if h == H - 1 and not last:
                            # commit-attached: only WAR consumers (slot
                            # reuse) key off these counts, and commit means
                            # the matmul's SBUF reads are done.
                            mm.then_inc(pe_sem, 1)
                    if last:
                        # the 16th inc gates the PSUM evacuation: drain so
                        # the systolic array has written PSUM before ACT
                        # reads it.
                        tensor.drain(fusable=False).then_inc(pe_sem, 1)

    return nc


def _get_nc():
    global _nc_cache
    if _nc_cache is None:
        _nc_cache = _build_nc()
    return _nc_cache


def _make_in_maps(keys, values, write_strengths):
    kf = np.ascontiguousarray(keys.reshape(B * S, FD))
    vf = np.ascontiguousarray(values.reshape(B * S, FD))
    wf = np.asarray(write_strengths).reshape(B * S)
    in_maps = []
    for c in range(N_CORES):
        sl = slice(c * NS, (c + 1) * NS)
        rho_pad = np.zeros((P, P), np.float32)
        rho_pad[:, :CHUNKS] = wf[sl].reshape(CHUNKS, P).T
        in_maps.append(
            {
                "keys": np.ascontiguousarray(kf[sl]),
                "values": np.ascontiguousarray(vf[sl]),
                "rho": rho_pad,
            }
        )
    return in_maps


def _run(in_maps, **kwargs):
    from concourse.bass_utils import run_bass_kernel_spmd

    nc = _get_nc()
    return run_bass_kernel_spmd(nc, in_maps, core_ids=list(range(N_CORES)), **kwargs)


def _assemble(memory, results):
    parts = np.stack([r["delta"] for r in results], axis=0)  # (8, 64, 1024)
    delta = parts.sum(axis=0, dtype=np.float64)  # (64, 1024) in [k, h*64+v]
    delta_hkv = delta.reshape(Dk, H, Dv).transpose(1, 0, 2)  # (H, Dk, Dv)
    out = DECAY * np.asarray(memory, dtype=np.float64) + delta_hkv
    return out.astype(np.float32)


def kernel(memory, keys, values, write_strengths):
    memory = np.asarray(memory, dtype=np.float32)
    keys = np.asarray(keys, dtype=np.float32)
    values = np.asarray(values, dtype=np.float32)
    write_strengths = np.asarray(write_strengths, dtype=np.float32)

    in_maps = _make_in_maps(keys, values, write_strengths)
    res = _run(in_maps)
    return _assemble(memory, res.results)


if __name__ == "__main__":
    rng = np.random.default_rng(0)
    mem = rng.standard_normal((H, Dk, Dv), dtype=np.float32)
    k = rng.standard_normal((B, S, H, Dk), dtype=np.float32)
    v = rng.standard_normal((B, S, H, Dv), dtype=np.float32)
    w = rng.random((B, S), dtype=np.float32)
    out = kernel(mem, k, v, w)
    ref = DECAY * mem + np.einsum(
        "bs,bshk,bshv->hkv", w.astype(np.float64), k.astype(np.float64), v.astype(np.float64)
    )
    err = np.abs(out - ref).max() / np.abs(ref).max()
    print("self-check rel err:", err)
